# revision 5
# baseline (speedup 1.0000x reference)
"""Trainium2 Bass kernel for the histogram_binning problem.

Math (per batch sample b):
  h = x[b] viewed as [C, N]  (C=2208 channels, N=196 positions)
  z[n, k] = sum_c h[c, n] * W[k, c] + bias[k]          (K=200 classes)
  max_val[n]  = max_k softmax(z[n,:]) = 1 / sum_k exp(z[n,k] - zmax[n])
  max_ids[n]  = argmax_k z[n, :]
  norm = max_val / ||max_val||_2
  p_r[k] = (sum_{n: ids[n]=k} max_val[n]) / L1   (L2 scale cancels under L1 norm)
  out[c, n] = x[c, n] * (1 + norm[n])

Distribution: pure data parallel, batch 64 -> 8 cores x 8 samples.

Implementation notes:
 - x is host-padded [C=2208] -> [CP=2304 = 18*128] rows; flat row 2208 is all
   ones and W row 2208 is fc_b, folding the bias add into the contraction.
 - Channel c maps to (partition p, chunk j) = (c // 18, c % 18) so the x DMA
   per sample is one fully contiguous 1.77MB transfer (14KB per partition).
   The weights are host-permuted the same way, so matmul j contracts the
   stride-18 channel subset {18p + j}; summed over j this is the full C sum.
 - argmax one-hot is computed as (z - zmax == 0) elementwise on the PSUM tile.
 - The scatter-add histogram matmul mv^T @ [mask | mv | ones] also yields
   sum(mv^2) (for the L2 norm) and sum(mv) (= L1 of the histogram) for free.
 - 1/sqrt(ssq) is computed as exp(-0.5*ln(ssq)): ln/exp/copy/square live in
   one ACT table set, avoiding the ~2.7us-per-swap sqrt table thrash.
"""

import numpy as np

import concourse.bass as bass
import concourse.bacc as bacc
import concourse.mybir as mybir
import concourse.tile as tile
from concourse.bass_utils import run_bass_kernel_spmd
from concourse.masks import make_identity

F32 = mybir.dt.float32

B = 64
C = 2208
CP = 2304            # padded channel dim: 18 * 128 (flat row 2208 = bias ones)
H = W = 14
N = H * W            # 196
K = 200
KE = K + 2           # mask cols: [one-hot(200) | mv | ones]
NCORES = 8
BPC = B // NCORES    # 8 samples per core
CT = CP // 128       # 18 contraction chunks
NT = ((0, 128), (128, 68))   # (offset, size) tiles of N=196
DVE_J = 11           # final multiply: chunks [0, DVE_J) on DVE, rest on gpsimd


def _build_nc() -> bass.Bass:
    nc = bacc.Bacc(None, target_bir_lowering=False, debug=False)
    x_d = nc.declare_dram_parameter("xs", [BPC, 128, CT * N], F32, isOutput=False)
    wp_d = nc.declare_dram_parameter("wp", [128, CT * K], F32, isOutput=False)
    out_d = nc.declare_dram_parameter("yo", [BPC, 128, CT * N], F32, isOutput=True)
    pr_d = nc.declare_dram_parameter("pr", [BPC, K], F32, isOutput=True)

    with tile.TileContext(nc) as tc:
        with (
            tc.tile_pool(name="consts", bufs=1) as consts,
            tc.tile_pool(name="xpool", bufs=1) as xpool,
            tc.tile_pool(name="maskp", bufs=4) as maskp,
            tc.tile_pool(name="escr", bufs=3) as escr,
            tc.tile_pool(name="stats", bufs=6) as stats,
            tc.tile_pool(name="brow", bufs=3) as brow,
            tc.tile_pool(name="tinyp", bufs=3) as tinyp,
            tc.tile_pool(name="bcsb", bufs=3) as bcsb,
            tc.tile_pool(name="psz", bufs=4, space="PSUM") as psz_pool,
            tc.tile_pool(name="pspr", bufs=2, space="PSUM") as pspr_pool,
            tc.tile_pool(name="pstr", bufs=1, space="PSUM") as pstr_pool,
            tc.tile_pool(name="psbc", bufs=1, space="PSUM") as psbc_pool,
        ):
            # --- constants ---
            w_sb = consts.tile([128, CT, K], F32)
            nc.sync.dma_start(
                out=w_sb, in_=wp_d[:, :].rearrange("p (t k) -> p t k", t=CT)
            )
            ident = consts.tile([128, 128], F32)
            make_identity(nc, ident)
            ones_row = consts.tile([1, 128], F32)
            nc.gpsimd.memset(ones_row, 1.0)

            for b in range(BPC):
                # --- load x[b]: one contiguous 1.77MB DMA (14112B/partition) ---
                x_b = xpool.tile([128, CT, N], F32, tag=f"x{b}")
                nc.sync.dma_start(
                    out=x_b,
                    in_=x_d[b].rearrange("p (t n) -> p t n", t=CT),
                )

                pspr = pspr_pool.tile([1, KE], F32, tag="pr")
                pstr = pstr_pool.tile([1, N], F32, tag="tr")
                masks = []

                for i, (noff, nsz) in enumerate(NT):
                    # z = x^T W (+bias via ones row), accumulated over 18 chunks
                    psz = psz_pool.tile([128, K], F32, tag="z")
                    for t in range(CT):
                        nc.tensor.matmul(
                            psz[:nsz, :],
                            lhsT=x_b[:, t, noff : noff + nsz],
                            rhs=w_sb[:, t, :],
                            start=(t == 0),
                            stop=(t == CT - 1),
                        )
                    # negmax[n] = -max_k z
                    negmax = stats.tile([128, 1], F32, tag="negmax")
                    nc.vector.tensor_reduce(
                        out=negmax[:nsz],
                        in_=psz[:nsz, :],
                        axis=mybir.AxisListType.X,
                        op=mybir.AluOpType.max,
                        negate=True,
                    )
                    # sumexp[n] = sum_k exp(z - zmax)
                    e_scr = escr.tile([128, K], F32, tag="escr")
                    sumexp = stats.tile([128, 1], F32, tag="sumexp")
                    nc.scalar.activation(
                        out=e_scr[:nsz],
                        in_=psz[:nsz, :],
                        func=mybir.ActivationFunctionType.Exp,
                        bias=negmax[:nsz],
                        scale=1.0,
                        accum_out=sumexp[:nsz],
                    )
                    # mask = [one-hot argmax (z + negmax == 0) | mv | ones]
                    mask = maskp.tile([128, KE], F32, tag="mask")
                    nc.vector.tensor_scalar(
                        mask[:nsz, :K],
                        psz[:nsz, :],
                        negmax[:nsz],
                        0.0,
                        op0=mybir.AluOpType.add,
                        op1=mybir.AluOpType.is_equal,
                    )
                    # max_val[n] = 1 / sumexp  -> mask col 200
                    nc.vector.reciprocal(mask[:nsz, K : K + 1], sumexp[:nsz])
                    nc.gpsimd.memset(mask[:nsz, K + 1 : K + 2], 1.0)
                    masks.append(mask)
                    # p_r_raw | ssq | L1  =  mv^T @ [mask | mv | ones]
                    nc.tensor.matmul(
                        pspr[:, :],
                        lhsT=mask[:nsz, K : K + 1],
                        rhs=mask[:nsz, :],
                        start=(i == 0),
                        stop=(i == 1),
                    )
                    # transpose max_val column into a row [1, N]
                    nc.tensor.transpose(
                        pstr[0:1, noff : noff + nsz],
                        mask[:nsz, K : K + 1],
                        ident[:nsz, :nsz],
                    )

                # --- per-sample tail: normalize + broadcast + scale ---
                mvrow = brow.tile([1, N], F32, tag="mvrow")
                nc.scalar.copy(mvrow, pstr[0:1, :])
                # rl2 = 1/sqrt(ssq) = exp(-0.5 * ln(ssq)); ssq = pspr[0, 200]
                lssq = tinyp.tile([1, 1], F32, tag="lssq")
                nc.scalar.activation(
                    out=lssq,
                    in_=pspr[0:1, K : K + 1],
                    func=mybir.ActivationFunctionType.Ln,
                )
                rl2 = tinyp.tile([1, 1], F32, tag="rl2")
                nc.scalar.activation(
                    out=rl2,
                    in_=lssq,
                    func=mybir.ActivationFunctionType.Exp,
                    scale=-0.5,
                )
                # n1row = max_val_row * rl2 + 1  == 1 + norm
                n1row = brow.tile([1, N], F32, tag="n1row")
                nc.vector.tensor_scalar(
                    n1row,
                    mvrow,
                    rl2,
                    1.0,
                    op0=mybir.AluOpType.mult,
                    op1=mybir.AluOpType.add,
                )
                # broadcast row to all 128 partitions: ones[1,128]^T @ n1row[1,N]
                psbc = psbc_pool.tile([128, N], F32, tag="bc")
                nc.tensor.matmul(psbc, lhsT=ones_row, rhs=n1row)
                bc_sb = bcsb.tile([128, N], F32, tag="bcsb")
                nc.vector.tensor_copy(bc_sb, psbc)

                # out = x * (1 + norm), in place; split across DVE and gpsimd
                nc.vector.tensor_tensor(
                    x_b[:, :DVE_J, :],
                    x_b[:, :DVE_J, :],
                    bc_sb[:, None, :].to_broadcast((128, DVE_J, N)),
                    op=mybir.AluOpType.mult,
                )
                nc.gpsimd.tensor_tensor(
                    x_b[:, DVE_J:, :],
                    x_b[:, DVE_J:, :],
                    bc_sb[:, None, :].to_broadcast((128, CT - DVE_J, N)),
                    op=mybir.AluOpType.mult,
                )
                nc.sync.dma_start(
                    out=out_d[b].rearrange("p (t n) -> p t n", t=CT),
                    in_=x_b,
                )

                # --- p_r row: L1 normalize (L1 = pspr[0, 201]) and store ---
                rl1 = tinyp.tile([1, 1], F32, tag="rl1")
                nc.vector.reciprocal(rl1, pspr[0:1, K + 1 : K + 2])
                pr_sb = brow.tile([1, K], F32, tag="prsb")
                nc.vector.tensor_scalar_mul(pr_sb, pspr[0:1, :K], rl1)
                nc.sync.dma_start(out=pr_d[b : b + 1, :], in_=pr_sb)

    nc.compile()
    return nc


_NC = None


def _get_nc():
    global _NC
    if _NC is None:
        _NC = _build_nc()
    return _NC


def _pack_weights(fc_w: np.ndarray, fc_b: np.ndarray) -> np.ndarray:
    """[128, CT*K]: W^T padded to CP rows (row 2208 = fc_b), then permuted so
    partition p chunk j holds channel c = 18p + j."""
    wp = np.zeros((CP, K), dtype=np.float32)
    wp[:C] = fc_w.astype(np.float32, copy=False).T
    wp[C] = fc_b.astype(np.float32, copy=False)
    return np.ascontiguousarray(wp.reshape(128, CT * K))


def _pad_x(x: np.ndarray) -> np.ndarray:
    """[B, 128, CT*N]: x rows, a ones row at flat 2208, zeros to 2304."""
    xp = np.zeros((B, CP, N), dtype=np.float32)
    xp[:, :C] = x.reshape(B, C, N)
    xp[:, C] = 1.0
    return xp.reshape(B, 128, CT * N)


def _run(x, fc_w, fc_b, flag, trace=False, trace_cores=None):
    x = np.asarray(x, dtype=np.float32)
    xp = _pad_x(x)
    wp = _pack_weights(np.asarray(fc_w), np.asarray(fc_b))
    in_maps = [
        {"xs": np.ascontiguousarray(xp[i * BPC : (i + 1) * BPC]), "wp": wp}
        for i in range(NCORES)
    ]
    nc = _get_nc()
    res = run_bass_kernel_spmd(
        nc,
        in_maps,
        core_ids=list(range(NCORES)),
        trace=trace,
        **({"trace_cores": trace_cores} if trace_cores else {}),
    )
    out = np.concatenate(
        [r["yo"].reshape(BPC, CP, N)[:, :C] for r in res.results], axis=0
    )
    out = out.reshape(B, C, H, W)
    p_r = np.concatenate([r["pr"] for r in res.results], axis=0)
    if not int(np.asarray(flag)):
        p_r = np.zeros_like(p_r)
    return (out, p_r), res


def kernel(x, fc_w, fc_b, flag):
    (out, p_r), _ = _run(x, fc_w, fc_b, flag)
    return out, p_r


# revision 7
# speedup vs baseline: 1.1497x; 1.1497x over previous
"""Trainium2 Bass kernel for the histogram_binning problem.

Math (per batch sample b):
  h = x[b] viewed as [C, N]  (C=2208 channels, N=196 positions)
  z[n, k] = sum_c h[c, n] * W[k, c] + bias[k]          (K=200 classes)
  max_val[n]  = max_k softmax(z[n,:]) = 1 / sum_k exp(z[n,k] - zmax[n])
  max_ids[n]  = argmax_k z[n, :]
  norm = max_val / ||max_val||_2
  p_r[k] = (sum_{n: ids[n]=k} max_val[n]) / L1   (L2 scale cancels under L1 norm)
  out[c, n] = x[c, n] * (1 + norm[n])

Distribution: pure data parallel, batch 64 -> 8 cores x 8 samples.

Implementation notes:
 - x is host-padded [C=2208] -> [CP=2304 = 18*128] rows; flat row 2208 is all
   ones and W row 2208 is fc_b, folding the bias add into the contraction.
 - Channel c maps to (partition p, chunk j) = (c // 18, c % 18) so the x DMA
   per sample is one fully contiguous 1.77MB transfer (14KB per partition).
   The weights are host-permuted the same way, so matmul j contracts the
   stride-18 channel subset {18p + j}; summed over j this is the full C sum.
 - argmax one-hot is computed as (z - zmax == 0) elementwise on the PSUM tile.
 - The scatter-add histogram matmul mv^T @ [mask | mv | ones] also yields
   sum(mv^2) (for the L2 norm) and sum(mv) (= L1 of the histogram) for free.
 - 1/sqrt(ssq) is computed as exp(-0.5*ln(ssq)): ln/exp/copy/square live in
   one ACT table set, avoiding the ~2.7us-per-swap sqrt table thrash.
"""

import numpy as np

import concourse.bass as bass
import concourse.bacc as bacc
import concourse.mybir as mybir
import concourse.tile as tile
from concourse.bass_utils import run_bass_kernel_spmd
from concourse.masks import make_identity

F32 = mybir.dt.float32

B = 64
C = 2208
CP = 2304            # padded channel dim: 18 * 128 (flat row 2208 = bias ones)
H = W = 14
N = H * W            # 196
K = 200
KE = K + 2           # mask cols: [one-hot(200) | mv | ones]
NCORES = 8
BPC = B // NCORES    # 8 samples per core
CT = CP // 128       # 18 contraction chunks
NT = ((0, 128), (128, 68))   # (offset, size) tiles of N=196
DVE_J = 14           # final multiply: chunks [0, DVE_J) on DVE, rest on gpsimd


def _pin_act_table_set():
    """Steer Bacc's act-table-load pass to one set that covers every
    activation we use (exp, ln, copy, square, identity), so the kernel does a
    single ACT_TABLE_LOAD instead of thrashing ~2.7us swaps between the
    default per-function first-match sets. Set ids/order are preserved; we
    only hide functions from the other sets."""
    import concourse.hw_specs as hw_specs

    if getattr(hw_specs.get_activation_tables, "_pinned", False):
        return
    orig = hw_specs.get_activation_tables

    @hw_specs.functools.cache
    def pinned(module_arch):
        tables = dict(orig(module_arch))
        keep = "natural_log_exp_and_others"
        if keep in tables:
            ours = {
                mybir.ActivationFunctionType.Exp,
                mybir.ActivationFunctionType.Ln,
                mybir.ActivationFunctionType.Copy,
                mybir.ActivationFunctionType.Identity,
                mybir.ActivationFunctionType.Square,
            }
            if ours <= tables[keep]:
                tables = {
                    name: (fns if name == keep else fns - ours)
                    for name, fns in tables.items()
                }
        return tables

    pinned._pinned = True
    hw_specs.get_activation_tables = pinned
    import concourse.bacc as _bacc_mod

    _bacc_mod.get_activation_tables = pinned


def _build_nc() -> bass.Bass:
    _pin_act_table_set()
    nc = bacc.Bacc(None, target_bir_lowering=False, debug=False)
    x_d = nc.declare_dram_parameter("xs", [BPC, 128, CT * N], F32, isOutput=False)
    wp_d = nc.declare_dram_parameter("wp", [128, CT * K], F32, isOutput=False)
    out_d = nc.declare_dram_parameter("yo", [BPC, 128, CT * N], F32, isOutput=True)
    pr_d = nc.declare_dram_parameter("pr", [BPC, K], F32, isOutput=True)

    with tile.TileContext(nc) as tc:
        with (
            tc.tile_pool(name="consts", bufs=1) as consts,
            tc.tile_pool(name="xpool", bufs=1) as xpool,
            tc.tile_pool(name="maskp", bufs=4) as maskp,
            tc.tile_pool(name="escr", bufs=3) as escr,
            tc.tile_pool(name="stats", bufs=6) as stats,
            tc.tile_pool(name="brow", bufs=3) as brow,
            tc.tile_pool(name="tinyp", bufs=3) as tinyp,
            tc.tile_pool(name="bcsb", bufs=3) as bcsb,
            tc.tile_pool(name="psz", bufs=4, space="PSUM") as psz_pool,
            tc.tile_pool(name="pspr", bufs=2, space="PSUM") as pspr_pool,
            tc.tile_pool(name="pstr", bufs=1, space="PSUM") as pstr_pool,
            tc.tile_pool(name="psbc", bufs=1, space="PSUM") as psbc_pool,
        ):
            # --- constants ---
            w_sb = consts.tile([128, CT, K], F32)
            nc.sync.dma_start(
                out=w_sb, in_=wp_d[:, :].rearrange("p (t k) -> p t k", t=CT)
            )
            ident = consts.tile([128, 128], F32)
            make_identity(nc, ident)
            ones_row = consts.tile([1, 128], F32)
            nc.gpsimd.memset(ones_row, 1.0)

            for b in range(BPC):
                # --- load x[b]: one contiguous 1.77MB DMA (14112B/partition) ---
                x_b = xpool.tile([128, CT, N], F32, tag=f"x{b}")
                nc.sync.dma_start(
                    out=x_b,
                    in_=x_d[b].rearrange("p (t n) -> p t n", t=CT),
                )

                pspr = pspr_pool.tile([1, KE], F32, tag="pr")
                pstr = pstr_pool.tile([1, N], F32, tag="tr")
                masks = []

                for i, (noff, nsz) in enumerate(NT):
                    # z = x^T W (+bias via ones row), accumulated over 18 chunks
                    psz = psz_pool.tile([128, K], F32, tag="z")
                    for t in range(CT):
                        nc.tensor.matmul(
                            psz[:nsz, :],
                            lhsT=x_b[:, t, noff : noff + nsz],
                            rhs=w_sb[:, t, :],
                            start=(t == 0),
                            stop=(t == CT - 1),
                        )
                    # negmax[n] = -max_k z
                    negmax = stats.tile([128, 1], F32, tag="negmax")
                    nc.vector.tensor_reduce(
                        out=negmax[:nsz],
                        in_=psz[:nsz, :],
                        axis=mybir.AxisListType.X,
                        op=mybir.AluOpType.max,
                        negate=True,
                    )
                    # sumexp[n] = sum_k exp(z - zmax)
                    e_scr = escr.tile([128, K], F32, tag="escr")
                    sumexp = stats.tile([128, 1], F32, tag="sumexp")
                    nc.scalar.activation(
                        out=e_scr[:nsz],
                        in_=psz[:nsz, :],
                        func=mybir.ActivationFunctionType.Exp,
                        bias=negmax[:nsz],
                        scale=1.0,
                        accum_out=sumexp[:nsz],
                    )
                    # mask = [one-hot argmax (z + negmax == 0) | mv | ones]
                    mask = maskp.tile([128, KE], F32, tag="mask")
                    nc.vector.tensor_scalar(
                        mask[:nsz, :K],
                        psz[:nsz, :],
                        negmax[:nsz],
                        0.0,
                        op0=mybir.AluOpType.add,
                        op1=mybir.AluOpType.is_equal,
                    )
                    # max_val[n] = 1 / sumexp  -> mask col 200
                    nc.vector.reciprocal(mask[:nsz, K : K + 1], sumexp[:nsz])
                    nc.gpsimd.memset(mask[:nsz, K + 1 : K + 2], 1.0)
                    masks.append(mask)
                    # p_r_raw | ssq | L1  =  mv^T @ [mask | mv | ones]
                    nc.tensor.matmul(
                        pspr[:, :],
                        lhsT=mask[:nsz, K : K + 1],
                        rhs=mask[:nsz, :],
                        start=(i == 0),
                        stop=(i == 1),
                    )
                    # transpose max_val column into a row [1, N]
                    nc.tensor.transpose(
                        pstr[0:1, noff : noff + nsz],
                        mask[:nsz, K : K + 1],
                        ident[:nsz, :nsz],
                    )

                # --- per-sample tail: normalize + broadcast + scale ---
                mvrow = brow.tile([1, N], F32, tag="mvrow")
                nc.scalar.copy(mvrow, pstr[0:1, :])
                # rl2 = 1/sqrt(ssq) = exp(-0.5 * ln(ssq)); ssq = pspr[0, 200]
                lssq = tinyp.tile([1, 1], F32, tag="lssq")
                nc.scalar.activation(
                    out=lssq,
                    in_=pspr[0:1, K : K + 1],
                    func=mybir.ActivationFunctionType.Ln,
                )
                rl2 = tinyp.tile([1, 1], F32, tag="rl2")
                nc.scalar.activation(
                    out=rl2,
                    in_=lssq,
                    func=mybir.ActivationFunctionType.Exp,
                    scale=-0.5,
                )
                # n1row = max_val_row * rl2 + 1  == 1 + norm
                n1row = brow.tile([1, N], F32, tag="n1row")
                nc.vector.tensor_scalar(
                    n1row,
                    mvrow,
                    rl2,
                    1.0,
                    op0=mybir.AluOpType.mult,
                    op1=mybir.AluOpType.add,
                )
                # broadcast row to all 128 partitions: ones[1,128]^T @ n1row[1,N]
                psbc = psbc_pool.tile([128, N], F32, tag="bc")
                nc.tensor.matmul(psbc, lhsT=ones_row, rhs=n1row)
                bc_sb = bcsb.tile([128, N], F32, tag="bcsb")
                nc.vector.tensor_copy(bc_sb, psbc)

                # out = x * (1 + norm), in place; split across DVE and gpsimd
                nc.vector.tensor_tensor(
                    x_b[:, :DVE_J, :],
                    x_b[:, :DVE_J, :],
                    bc_sb[:, None, :].to_broadcast((128, DVE_J, N)),
                    op=mybir.AluOpType.mult,
                )
                nc.gpsimd.tensor_tensor(
                    x_b[:, DVE_J:, :],
                    x_b[:, DVE_J:, :],
                    bc_sb[:, None, :].to_broadcast((128, CT - DVE_J, N)),
                    op=mybir.AluOpType.mult,
                )
                nc.sync.dma_start(
                    out=out_d[b].rearrange("p (t n) -> p t n", t=CT),
                    in_=x_b,
                )

                # --- p_r row: L1 normalize (L1 = pspr[0, 201]) and store ---
                rl1 = tinyp.tile([1, 1], F32, tag="rl1")
                nc.vector.reciprocal(rl1, pspr[0:1, K + 1 : K + 2])
                pr_sb = brow.tile([1, K], F32, tag="prsb")
                nc.vector.tensor_scalar_mul(pr_sb, pspr[0:1, :K], rl1)
                nc.sync.dma_start(out=pr_d[b : b + 1, :], in_=pr_sb)

    nc.compile()
    return nc


_NC = None


def _get_nc():
    global _NC
    if _NC is None:
        _NC = _build_nc()
    return _NC


def _pack_weights(fc_w: np.ndarray, fc_b: np.ndarray) -> np.ndarray:
    """[128, CT*K]: W^T padded to CP rows (row 2208 = fc_b), then permuted so
    partition p chunk j holds channel c = 18p + j."""
    wp = np.zeros((CP, K), dtype=np.float32)
    wp[:C] = fc_w.astype(np.float32, copy=False).T
    wp[C] = fc_b.astype(np.float32, copy=False)
    return np.ascontiguousarray(wp.reshape(128, CT * K))


def _pad_x(x: np.ndarray) -> np.ndarray:
    """[B, 128, CT*N]: x rows, a ones row at flat 2208, zeros to 2304."""
    xp = np.zeros((B, CP, N), dtype=np.float32)
    xp[:, :C] = x.reshape(B, C, N)
    xp[:, C] = 1.0
    return xp.reshape(B, 128, CT * N)


def _run(x, fc_w, fc_b, flag, trace=False, trace_cores=None):
    x = np.asarray(x, dtype=np.float32)
    xp = _pad_x(x)
    wp = _pack_weights(np.asarray(fc_w), np.asarray(fc_b))
    in_maps = [
        {"xs": np.ascontiguousarray(xp[i * BPC : (i + 1) * BPC]), "wp": wp}
        for i in range(NCORES)
    ]
    nc = _get_nc()
    res = run_bass_kernel_spmd(
        nc,
        in_maps,
        core_ids=list(range(NCORES)),
        trace=trace,
        **({"trace_cores": trace_cores} if trace_cores else {}),
    )
    out = np.concatenate(
        [r["yo"].reshape(BPC, CP, N)[:, :C] for r in res.results], axis=0
    )
    out = out.reshape(B, C, H, W)
    p_r = np.concatenate([r["pr"] for r in res.results], axis=0)
    if not int(np.asarray(flag)):
        p_r = np.zeros_like(p_r)
    return (out, p_r), res


def kernel(x, fc_w, fc_b, flag):
    (out, p_r), _ = _run(x, fc_w, fc_b, flag)
    return out, p_r


# revision 8
# speedup vs baseline: 1.1785x; 1.0251x over previous
"""Trainium2 Bass kernel for the histogram_binning problem.

Math (per batch sample b):
  h = x[b] viewed as [C, N]  (C=2208 channels, N=196 positions)
  z[n, k] = sum_c h[c, n] * W[k, c] + bias[k]          (K=200 classes)
  max_val[n]  = max_k softmax(z[n,:]) = 1 / sum_k exp(z[n,k] - zmax[n])
  max_ids[n]  = argmax_k z[n, :]
  norm = max_val / ||max_val||_2
  p_r[k] = (sum_{n: ids[n]=k} max_val[n]) / L1   (L2 scale cancels under L1 norm)
  out[c, n] = x[c, n] * (1 + norm[n])

Distribution: pure data parallel, batch 64 -> 8 cores x 8 samples.

Implementation notes:
 - x is host-padded [C=2208] -> [CP=2304 = 18*128] rows; flat row 2208 is all
   ones and W row 2208 is fc_b, folding the bias add into the contraction.
 - Channel c maps to (partition p, chunk j) = (c // 18, c % 18) so the x DMA
   per sample is one fully contiguous 1.77MB transfer (14KB per partition).
   The weights are host-permuted the same way, so matmul j contracts the
   stride-18 channel subset {18p + j}; summed over j this is the full C sum.
 - argmax one-hot is computed as (z - zmax == 0) elementwise on the PSUM tile.
 - The scatter-add histogram matmul mv^T @ [mask | mv | ones] also yields
   sum(mv^2) (for the L2 norm) and sum(mv) (= L1 of the histogram) for free.
 - 1/sqrt(ssq) is computed as exp(-0.5*ln(ssq)): ln/exp/copy/square live in
   one ACT table set, avoiding the ~2.7us-per-swap sqrt table thrash.
"""

import numpy as np

import concourse.bass as bass
import concourse.bacc as bacc
import concourse.mybir as mybir
import concourse.tile as tile
from concourse.bass_utils import run_bass_kernel_spmd
from concourse.masks import make_identity

F32 = mybir.dt.float32

B = 64
C = 2208
CP = 2304            # padded channel dim: 18 * 128 (flat row 2208 = bias ones)
H = W = 14
N = H * W            # 196
K = 200
KE = K + 2           # mask cols: [one-hot(200) | mv | ones]
NCORES = 8
BPC = B // NCORES    # 8 samples per core
CT = CP // 128       # 18 contraction chunks
NT = ((0, 128), (128, 68))   # (offset, size) tiles of N=196
DVE_J = 16           # final multiply: chunks [0, DVE_J) on DVE, rest on gpsimd


def _pin_act_table_set():
    """Steer Bacc's act-table-load pass to one set that covers every
    activation we use (exp, ln, copy, square, identity), so the kernel does a
    single ACT_TABLE_LOAD instead of thrashing ~2.7us swaps between the
    default per-function first-match sets. Set ids/order are preserved; we
    only hide functions from the other sets."""
    import concourse.hw_specs as hw_specs

    if getattr(hw_specs.get_activation_tables, "_pinned", False):
        return
    orig = hw_specs.get_activation_tables

    @hw_specs.functools.cache
    def pinned(module_arch):
        tables = dict(orig(module_arch))
        keep = "natural_log_exp_and_others"
        if keep in tables:
            ours = {
                mybir.ActivationFunctionType.Exp,
                mybir.ActivationFunctionType.Ln,
                mybir.ActivationFunctionType.Copy,
                mybir.ActivationFunctionType.Identity,
                mybir.ActivationFunctionType.Square,
            }
            if ours <= tables[keep]:
                tables = {
                    name: (fns if name == keep else fns - ours)
                    for name, fns in tables.items()
                }
        return tables

    pinned._pinned = True
    hw_specs.get_activation_tables = pinned
    import concourse.bacc as _bacc_mod

    _bacc_mod.get_activation_tables = pinned


def _build_nc() -> bass.Bass:
    _pin_act_table_set()
    nc = bacc.Bacc(None, target_bir_lowering=False, debug=False)
    x_d = nc.declare_dram_parameter("xs", [BPC, 128, CT * N], F32, isOutput=False)
    wp_d = nc.declare_dram_parameter("wp", [128, CT * K], F32, isOutput=False)
    out_d = nc.declare_dram_parameter("yo", [BPC, 128, CT * N], F32, isOutput=True)
    pr_d = nc.declare_dram_parameter("pr", [BPC, K], F32, isOutput=True)

    with tile.TileContext(nc) as tc:
        with (
            tc.tile_pool(name="consts", bufs=1) as consts,
            tc.tile_pool(name="xpool", bufs=1) as xpool,
            tc.tile_pool(name="maskp", bufs=4) as maskp,
            tc.tile_pool(name="escr", bufs=3) as escr,
            tc.tile_pool(name="stats", bufs=6) as stats,
            tc.tile_pool(name="brow", bufs=3) as brow,
            tc.tile_pool(name="tinyp", bufs=3) as tinyp,
            tc.tile_pool(name="bcsb", bufs=3) as bcsb,
            tc.tile_pool(name="psz", bufs=4, space="PSUM") as psz_pool,
            tc.tile_pool(name="pspr", bufs=2, space="PSUM") as pspr_pool,
            tc.tile_pool(name="pstr", bufs=1, space="PSUM") as pstr_pool,
            tc.tile_pool(name="psbc", bufs=1, space="PSUM") as psbc_pool,
        ):
            # --- constants ---
            w_sb = consts.tile([128, CT, K], F32)
            wp_v = wp_d[:, :].rearrange("p (t k) -> p t k", t=CT)
            for wpc in range(0, CT, 6):
                nc.sync.dma_start(
                    out=w_sb[:, wpc : wpc + 6, :], in_=wp_v[:, wpc : wpc + 6, :]
                )
            ident = consts.tile([128, 128], F32)
            make_identity(nc, ident)
            ones_row = consts.tile([1, 128], F32)
            nc.gpsimd.memset(ones_row, 1.0)

            for b in range(BPC):
                # --- load x[b]: one contiguous 1.77MB DMA (14112B/partition) ---
                x_b = xpool.tile([128, CT, N], F32, tag=f"x{b}")
                x_v = x_d[b].rearrange("p (t n) -> p t n", t=CT)
                for xc in range(0, CT, 6):
                    nc.sync.dma_start(
                        out=x_b[:, xc : xc + 6, :], in_=x_v[:, xc : xc + 6, :]
                    )

                pspr = pspr_pool.tile([1, KE], F32, tag="pr")
                pstr = pstr_pool.tile([1, N], F32, tag="tr")
                masks = []

                for i, (noff, nsz) in enumerate(NT):
                    # z = x^T W (+bias via ones row), accumulated over 18 chunks
                    psz = psz_pool.tile([128, K], F32, tag="z")
                    for t in range(CT):
                        nc.tensor.matmul(
                            psz[:nsz, :],
                            lhsT=x_b[:, t, noff : noff + nsz],
                            rhs=w_sb[:, t, :],
                            start=(t == 0),
                            stop=(t == CT - 1),
                        )
                    # negmax[n] = -max_k z
                    negmax = stats.tile([128, 1], F32, tag="negmax")
                    nc.vector.tensor_reduce(
                        out=negmax[:nsz],
                        in_=psz[:nsz, :],
                        axis=mybir.AxisListType.X,
                        op=mybir.AluOpType.max,
                        negate=True,
                    )
                    # sumexp[n] = sum_k exp(z - zmax)
                    e_scr = escr.tile([128, K], F32, tag="escr")
                    sumexp = stats.tile([128, 1], F32, tag="sumexp")
                    nc.scalar.activation(
                        out=e_scr[:nsz],
                        in_=psz[:nsz, :],
                        func=mybir.ActivationFunctionType.Exp,
                        bias=negmax[:nsz],
                        scale=1.0,
                        accum_out=sumexp[:nsz],
                    )
                    # mask = [one-hot argmax (z + negmax == 0) | mv | ones]
                    mask = maskp.tile([128, KE], F32, tag="mask")
                    nc.vector.tensor_scalar(
                        mask[:nsz, :K],
                        psz[:nsz, :],
                        negmax[:nsz],
                        0.0,
                        op0=mybir.AluOpType.add,
                        op1=mybir.AluOpType.is_equal,
                    )
                    # max_val[n] = 1 / sumexp  -> mask col 200
                    nc.vector.reciprocal(mask[:nsz, K : K + 1], sumexp[:nsz])
                    nc.gpsimd.memset(mask[:nsz, K + 1 : K + 2], 1.0)
                    masks.append(mask)
                    # p_r_raw | ssq | L1  =  mv^T @ [mask | mv | ones]
                    nc.tensor.matmul(
                        pspr[:, :],
                        lhsT=mask[:nsz, K : K + 1],
                        rhs=mask[:nsz, :],
                        start=(i == 0),
                        stop=(i == 1),
                    )
                    # transpose max_val column into a row [1, N]
                    nc.tensor.transpose(
                        pstr[0:1, noff : noff + nsz],
                        mask[:nsz, K : K + 1],
                        ident[:nsz, :nsz],
                    )

                # --- per-sample tail: normalize + broadcast + scale ---
                mvrow = brow.tile([1, N], F32, tag="mvrow")
                nc.scalar.copy(mvrow, pstr[0:1, :])
                # rl2 = 1/sqrt(ssq) = exp(-0.5 * ln(ssq)); ssq = pspr[0, 200]
                lssq = tinyp.tile([1, 1], F32, tag="lssq")
                nc.scalar.activation(
                    out=lssq,
                    in_=pspr[0:1, K : K + 1],
                    func=mybir.ActivationFunctionType.Ln,
                )
                rl2 = tinyp.tile([1, 1], F32, tag="rl2")
                nc.scalar.activation(
                    out=rl2,
                    in_=lssq,
                    func=mybir.ActivationFunctionType.Exp,
                    scale=-0.5,
                )
                # n1row = max_val_row * rl2 + 1  == 1 + norm
                n1row = brow.tile([1, N], F32, tag="n1row")
                nc.vector.tensor_scalar(
                    n1row,
                    mvrow,
                    rl2,
                    1.0,
                    op0=mybir.AluOpType.mult,
                    op1=mybir.AluOpType.add,
                )
                # broadcast row to all 128 partitions: ones[1,128]^T @ n1row[1,N]
                psbc = psbc_pool.tile([128, N], F32, tag="bc")
                nc.tensor.matmul(psbc, lhsT=ones_row, rhs=n1row)
                bc_sb = bcsb.tile([128, N], F32, tag="bcsb")
                nc.vector.tensor_copy(bc_sb, psbc)

                # out = x * (1 + norm), in place; split across DVE and gpsimd
                nc.vector.tensor_tensor(
                    x_b[:, :DVE_J, :],
                    x_b[:, :DVE_J, :],
                    bc_sb[:, None, :].to_broadcast((128, DVE_J, N)),
                    op=mybir.AluOpType.mult,
                )
                nc.gpsimd.tensor_tensor(
                    x_b[:, DVE_J:, :],
                    x_b[:, DVE_J:, :],
                    bc_sb[:, None, :].to_broadcast((128, CT - DVE_J, N)),
                    op=mybir.AluOpType.mult,
                )
                out_v = out_d[b].rearrange("p (t n) -> p t n", t=CT)
                nc.sync.dma_start(out=out_v[:, :DVE_J, :], in_=x_b[:, :DVE_J, :])
                nc.sync.dma_start(out=out_v[:, DVE_J:, :], in_=x_b[:, DVE_J:, :])

                # --- p_r row: L1 normalize (L1 = pspr[0, 201]) and store ---
                rl1 = tinyp.tile([1, 1], F32, tag="rl1")
                nc.vector.reciprocal(rl1, pspr[0:1, K + 1 : K + 2])
                pr_sb = brow.tile([1, K], F32, tag="prsb")
                nc.vector.tensor_scalar_mul(pr_sb, pspr[0:1, :K], rl1)
                nc.sync.dma_start(out=pr_d[b : b + 1, :], in_=pr_sb)

    nc.compile()
    return nc


_NC = None


def _get_nc():
    global _NC
    if _NC is None:
        _NC = _build_nc()
    return _NC


def _pack_weights(fc_w: np.ndarray, fc_b: np.ndarray) -> np.ndarray:
    """[128, CT*K]: W^T padded to CP rows (row 2208 = fc_b), then permuted so
    partition p chunk j holds channel c = 18p + j."""
    wp = np.zeros((CP, K), dtype=np.float32)
    wp[:C] = fc_w.astype(np.float32, copy=False).T
    wp[C] = fc_b.astype(np.float32, copy=False)
    return np.ascontiguousarray(wp.reshape(128, CT * K))


def _pad_x(x: np.ndarray) -> np.ndarray:
    """[B, 128, CT*N]: x rows, a ones row at flat 2208, zeros to 2304."""
    xp = np.zeros((B, CP, N), dtype=np.float32)
    xp[:, :C] = x.reshape(B, C, N)
    xp[:, C] = 1.0
    return xp.reshape(B, 128, CT * N)


def _run(x, fc_w, fc_b, flag, trace=False, trace_cores=None):
    x = np.asarray(x, dtype=np.float32)
    xp = _pad_x(x)
    wp = _pack_weights(np.asarray(fc_w), np.asarray(fc_b))
    in_maps = [
        {"xs": np.ascontiguousarray(xp[i * BPC : (i + 1) * BPC]), "wp": wp}
        for i in range(NCORES)
    ]
    nc = _get_nc()
    res = run_bass_kernel_spmd(
        nc,
        in_maps,
        core_ids=list(range(NCORES)),
        trace=trace,
        **({"trace_cores": trace_cores} if trace_cores else {}),
    )
    out = np.concatenate(
        [r["yo"].reshape(BPC, CP, N)[:, :C] for r in res.results], axis=0
    )
    out = out.reshape(B, C, H, W)
    p_r = np.concatenate([r["pr"] for r in res.results], axis=0)
    if not int(np.asarray(flag)):
        p_r = np.zeros_like(p_r)
    return (out, p_r), res


def kernel(x, fc_w, fc_b, flag):
    (out, p_r), _ = _run(x, fc_w, fc_b, flag)
    return out, p_r
